# revision 16
# baseline (speedup 1.0000x reference)
"""Trainium2 kernel for nn_DependencyParsingNetwork.

Pipeline:
  * Host (numpy, float64): embedding lookup + 2-layer BiLSTM (inherently
    sequential, tiny) reduced to two length-N score vectors
    si = x @ w[:2H] + b, sj = x @ w[2H:], so that
    out[r, c] = tanh(si[r] + sj[c]) for c > r, else 0.
  * Device (raw Bass, 8 NeuronCores): the memory-bound N x N upper-triangular
    tanh outer-sum. Row-groups of 128 are dealt round-robin to cores
    (group g -> core g % 8, slot g // 8). Each slot computes only the strip
    starting at its own diagonal, so every core writes ~19 MB instead of 32 MB
    and the zero lower-triangle is never written on device.
    sj is split hi/lo into two bf16 rows so a single-pass K=2 matmul against
    a ones stationary reconstructs sj broadcast across all 128 partitions
    into PSUM (error ~2^-18; a fp32 matmul would need 2 PE passes). Each
    2048-col PSUM pair is consumed by every slot whose strip covers it: one
    ScalarE activation tanh(psum + si_bias) -> SBUF per slot, the 128x128
    diagonal tile masked on VectorE, then a 1 MiB DMA out. Hand-rolled
    semaphores (no TileContext) keep the preamble/tail small; an 8-deep SBUF
    chunk ring with per-slot DMA-completion sems keeps the output stream at
    HBM line rate (~420 GB/s observed per core).
    A single SPMD program works for all cores because each core receives
    sj shifted by 128*k, which makes strip offsets uniform (1024*i).
  * Host: scatter the 64 strips into the zero-initialized full output.
"""

import sys
import types

import numpy as np

N = 8192
H = 50
N_CORES = 8
N_SLOTS = 8  # row-groups of 128 per core
F32 = np.float32


# ---------------------------------------------------------------------------
# Host math: embedding + BiLSTM -> si, sj
# ---------------------------------------------------------------------------

def _lstm_pair(x, wf, hf, bf, wb, hb, bb, dtype):
    """Fwd+bwd LSTM over x [n, d] -> concat [n, 2H]. The two directions are
    batched into one python loop (fwd at step t, bwd at step n-1-t)."""
    n = x.shape[0]
    g_in = np.stack([
        x @ wf.T.astype(dtype) + bf.astype(dtype),
        x @ wb.T.astype(dtype) + bb.astype(dtype),
    ])  # [2, n, 4H]
    whhT = np.stack([hf.T.astype(dtype), hb.T.astype(dtype)])  # [2, H, 4H]
    h = np.zeros((2, H), dtype)
    c = np.zeros((2, H), dtype)
    out_f = np.empty((n, H), dtype)
    out_b = np.empty((n, H), dtype)
    for t in range(n):
        tb = n - 1 - t
        g = np.einsum("dk,dkg->dg", h, whhT)
        g[0] += g_in[0, t]
        g[1] += g_in[1, tb]
        i = 1.0 / (1.0 + np.exp(-g[:, :H]))
        f = 1.0 / (1.0 + np.exp(-g[:, H:2 * H]))
        gg = np.tanh(g[:, 2 * H:3 * H])
        o = 1.0 / (1.0 + np.exp(-g[:, 3 * H:]))
        c = f * c + i * gg
        h = o * np.tanh(c)
        out_f[t] = h[0]
        out_b[tb] = h[1]
    return np.concatenate([out_f, out_b], axis=1)


def _host_scores(inp, dtype=np.float64):
    emb = inp["emb"].astype(dtype)
    tok = np.asarray(inp["token_vector"]).reshape(-1).astype(np.int64)
    x = np.concatenate([emb[tok], inp["pos_vector"].astype(dtype)], axis=1)
    x = _lstm_pair(x, inp["w_ih_l0_fwd"], inp["w_hh_l0_fwd"], inp["b_l0_fwd"],
                   inp["w_ih_l0_bwd"], inp["w_hh_l0_bwd"], inp["b_l0_bwd"], dtype)
    x = _lstm_pair(x, inp["w_ih_l1_fwd"], inp["w_hh_l1_fwd"], inp["b_l1_fwd"],
                   inp["w_ih_l1_bwd"], inp["w_hh_l1_bwd"], inp["b_l1_bwd"], dtype)
    mlp_w = inp["mlp_w"].astype(dtype)
    si = x @ mlp_w[0, :2 * H] + inp["mlp_b"].astype(dtype)[0]
    sj = x @ mlp_w[0, 2 * H:]
    return si, sj


# ---------------------------------------------------------------------------
# Device kernel
# ---------------------------------------------------------------------------

def _install_ntff_hook_shim():
    """The agent image's antenv stub lacks axon_hooks; run_bass_kernel_spmd
    imports it when tracing is requested. Provide it if missing."""
    try:
        import antenv.axon_hooks  # noqa: F401
        return
    except ImportError:
        pass
    try:
        from trn_agent_boot.trn_boot import _ntff_profile_via_ctypes
        hook = _ntff_profile_via_ctypes("/opt/axon/libaxon_pjrt.so")
    except Exception:
        hook = None
    mod = types.ModuleType("antenv.axon_hooks")
    mod.get_axon_ntff_profile_hook = lambda: hook
    mod.set_axon_ntff_profile_hook = lambda h: None
    sys.modules["antenv.axon_hooks"] = mod


_NC_CACHE = {}


def _strip_width(i):
    return N - 1024 * i


def _chunks():
    """Global chunk order: (pair p, slot i, half?). Pair p covers local sj
    columns [2048p, 2048p+2048). Slot i uses pair p iff i <= 2p+1; the chunk
    is the pair's second half only when i == 2p+1, and the strip's first 128
    columns need the triangular mask when i in (2p, 2p+1)."""
    out = []
    for p in range(N // 2048):
        for i in range(min(2 * p + 1, N_SLOTS - 1) + 1):
            out.append((p, i, i == 2 * p + 1))
    return out


def _build_nc_raw():
    """Raw bacc implementation (no TileContext): hand-rolled semaphores,
    minimal preamble/tail."""
    if "nc_raw" in _NC_CACHE:
        return _NC_CACHE["nc_raw"]
    import concourse.bacc as bacc
    import concourse.mybir as mybir

    f32 = mybir.dt.float32
    bf16 = mybir.dt.bfloat16
    CW = 2048
    nc = bacc.Bacc("TRN2", target_bir_lowering=False, debug=False,
                   num_devices=N_CORES)
    # row 0: sj_hi ++ ones(128); row 1: sj_lo ++ ones(128)
    sj_d = nc.dram_tensor("sjhl", [2, N + 128], bf16, kind="ExternalInput")
    sim_d = nc.dram_tensor("simask", [128, N_SLOTS + 128], f32,
                           kind="ExternalInput")
    outs = [
        nc.dram_tensor(f"out{i}", [128, _strip_width(i)], f32,
                       kind="ExternalOutput")
        for i in range(N_SLOTS)
    ]

    chunks = _chunks()
    n_chunks = len(chunks)
    # masked chunk ordinal (prefix count) for each chunk index
    masked = [(i == 2 * p or half) for (p, i, half) in chunks]
    mord = [sum(masked[:n + 1]) for n in range(n_chunks)]
    cum_chunks = []  # chunks through pair p
    c = 0
    for p in range(N // CW):
        c += min(2 * p + 1, N_SLOTS - 1) + 1
        cum_chunks.append(c)

    NCH = 8  # ch ring depth

    with (
        nc.sbuf_tensor("sjhl_sb", [2, N + 128], bf16) as sj_sb,
        nc.sbuf_tensor("simask_sb", [128, N_SLOTS + 128], f32) as sim_sb,
        nc.sbuf_tensor("ch_sb", [128, NCH * CW], f32) as ch_sb,
        nc.psum_tensor("ps0", [128, CW], f32) as ps0,
        nc.psum_tensor("ps1", [128, CW], f32) as ps1,
    ):
        pss = [ps0, ps1]
        s_in_sj = nc.alloc_semaphore("s_in_sj")
        s_in_sim = nc.alloc_semaphore("s_in_sim")
        s_mm = nc.alloc_semaphore("s_mm")
        s_act = nc.alloc_semaphore("s_act")
        s_dve = nc.alloc_semaphore("s_dve")
        # per-ch-ring-slot DMA completion sems: slot j's DMAs are serialized
        # by the ACT->DMA->ACT chain, so each sem unambiguously counts its
        # own slot's completed DMAs (a shared counter would interleave
        # per-SDMA-engine increments of concurrent DMAs).
        s_slot = [nc.alloc_semaphore(f"s_slot{j}") for j in range(8)]
        my_sems = [s_in_sj, s_in_sim, s_mm, s_act, s_dve] + s_slot

        # Sems are zero at program start: the NEFF-level teardown emitted by
        # the compiler resets the whole semaphore space at the end of every
        # NEFF, so (like Tile programs) we rely on that convention instead of
        # spending a clear + all-engine barrier before the first DMA.
        del my_sems

        # Work records: one per (ACT, DMA) unit. Chunk 0 is split 512+1536 so
        # the first output DMA launches after a single matmul. Fields:
        # p: psum pair, i: slot, src: cols within pss[p%2], ch: (ring slot,
        # lo, hi) within that slot's ch region, dst: cols within out_i,
        # mm: s_mm threshold, msk: apply triangular mask (dst starts at the
        # strip diagonal in every masked record).
        recs = []
        for n, (p, i, half) in enumerate(chunks):
            sl = n % NCH
            if n == 0:
                recs.append((p, i, (0, 512), (sl, 0, 512), (0, 512), 1, True, n))
                recs.append((p, i, (512, CW), (sl, 512, CW), (512, CW), 4, False, n))
            elif half:
                recs.append((p, i, (1024, CW), (sl, 0, 1024), (0, 1024),
                             4 * (p + 1), True, n))
            else:
                off = CW * p - 1024 * i
                recs.append((p, i, (0, CW), (sl, 0, CW), (off, off + CW),
                             4 * (p + 1), i == 2 * p, n))

        # s_slot[s] DMA-increments contributed by chunks before chunk n
        dmas_of_chunk = [sum(1 for r in recs if r[7] == n) for n in range(n_chunks)]
        # ACT of chunk n (n>=NCH) must see all DMAs of chunks {n%NCH, .., n-NCH}
        act_slot_need = [0] * n_chunks
        for n in range(NCH, n_chunks):
            act_slot_need[n] = 16 * sum(dmas_of_chunk[m] for m in range(n % NCH, n - NCH + 1, NCH))
        mask_ord = [sum(1 for r2 in recs[:r + 1] if r2[6]) for r in range(len(recs))]
        recs_through_pair = [sum(1 for r in recs if r[0] <= p) for p in range(N // CW)]
        slot_total = [16 * sum(1 for r in recs if r[3][0] == j) for j in range(NCH)]

        with nc.Block() as block:

            @block.sync
            def _(sync):
                sync.dma_start(sj_sb.ap(), sj_d[:]).then_inc(s_in_sj, 16)
                sync.dma_start(sim_sb.ap(), sim_d[:]).then_inc(s_in_sim, 16)
                for r, (p, i, srcc, (sl, clo, chi), (dlo, dhi), mm, msk, n) in enumerate(recs):
                    if msk:
                        sync.wait_ge(s_dve, mask_ord[r])
                    else:
                        sync.wait_ge(s_act, r + 1)
                    sync.dma_start(
                        outs[i][:, dlo:dhi],
                        ch_sb[:, sl * CW + clo:sl * CW + chi],
                    ).then_inc(s_slot[sl], 16)
                for j in range(NCH):
                    sync.wait_ge(s_slot[j], slot_total[j])

            @block.tensor
            def _(tensor):
                tensor.wait_ge(s_in_sj, 16)
                for p in range(N // CW):
                    if p >= 2:
                        tensor.wait_ge(s_act, recs_through_pair[p - 2])
                    for q in range(CW // 512):
                        lo = CW * p + 512 * q
                        tensor.matmul(
                            pss[p % 2][:, 512 * q:512 * (q + 1)],
                            sj_sb[0:2, N:N + 128],
                            sj_sb[0:2, lo:lo + 512],
                        ).then_inc(s_mm, 1)

            @block.scalar
            def _(scalar):
                scalar.wait_ge(s_in_sim, 16)
                prev_chunk = -1
                for r, (p, i, (slo, shi), (sl, clo, chi), dst, mm, msk, n) in enumerate(recs):
                    scalar.wait_ge(s_mm, mm)
                    if n != prev_chunk and act_slot_need[n]:
                        scalar.wait_ge(s_slot[sl], act_slot_need[n])
                    prev_chunk = n
                    scalar.activation(
                        ch_sb[:, sl * CW + clo:sl * CW + chi],
                        pss[p % 2][:, slo:shi],
                        mybir.ActivationFunctionType.Tanh,
                        bias=sim_sb[:, i:i + 1],
                    ).then_inc(s_act, 1)

            @block.vector
            def _(vector):
                for r, (p, i, srcc, (sl, clo, chi), dst, mm, msk, n) in enumerate(recs):
                    if not msk:
                        continue
                    vector.wait_ge(s_act, r + 1)
                    vector.tensor_mul(
                        ch_sb[:, sl * CW:sl * CW + 128],
                        ch_sb[:, sl * CW:sl * CW + 128],
                        sim_sb[:, N_SLOTS:],
                    ).then_inc(s_dve, 1)

    nc.compile()
    _NC_CACHE["nc_raw"] = nc
    return nc


def _build_nc():
    if "nc" in _NC_CACHE:
        return _NC_CACHE["nc"]
    import concourse.bacc as bacc
    import concourse.mybir as mybir
    import concourse.tile as tile

    f32 = mybir.dt.float32
    bf16 = mybir.dt.bfloat16
    nc = bacc.Bacc("TRN2", target_bir_lowering=False, debug=False,
                   num_devices=N_CORES)
    # sj split into bf16 hi+lo rows so a single-pass K=2 bf16 matmul against
    # a ones stationary reconstructs sj (error ~2^-18) broadcast across all
    # 128 partitions: psum[p, c] = 1*hi[c] + 1*lo[c].
    sj_d = nc.dram_tensor("sjhl", [2, N], bf16, kind="ExternalInput")
    # si ([:, :8]) and the 128x128 triangular mask ([:, 8:136]) share one DMA
    sim_d = nc.dram_tensor("simask", [128, N_SLOTS + 128], f32,
                           kind="ExternalInput")
    outs = [
        nc.dram_tensor(f"out{i}", [128, _strip_width(i)], f32,
                       kind="ExternalOutput")
        for i in range(N_SLOTS)
    ]

    CW = 2048  # PSUM chunk width: 4 banks; one broadcast serves many slots

    with tile.TileContext(nc) as tc:
        with (
            tc.tile_pool(name="const", bufs=1) as cpool,
            tc.tile_pool(name="work", bufs=8) as wpool,
            tc.tile_pool(name="psum", bufs=2, space="PSUM") as ppool,
        ):
            sj_sb = cpool.tile([2, N], bf16)
            nc.sync.dma_start(sj_sb[:], sj_d[:])
            sim_sb = cpool.tile([128, N_SLOTS + 128], f32)
            nc.sync.dma_start(sim_sb[:], sim_d[:])
            si_sb = sim_sb[:, :N_SLOTS]
            mask_sb = sim_sb[:, N_SLOTS:]
            ones_sb = cpool.tile([2, 128], bf16)
            nc.gpsimd.memset(ones_sb[:], 1.0)

            # Pair p covers local sj columns [2048p, 2048p+2048): the 1024-col
            # ranges r=2p (first half) and r=2p+1 (second half). Slot i's strip
            # starts at local col 1024i, so slot i uses pair p iff i <= 2p+1.
            for p in range(N // CW):
                ps = ppool.tile([128, CW], f32, tag="ps")
                for q in range(CW // 512):
                    lo = CW * p + 512 * q
                    nc.tensor.matmul(
                        ps[:, 512 * q:512 * (q + 1)],
                        ones_sb[:, :],
                        sj_sb[0:2, lo:lo + 512],
                    )
                for i in range(min(2 * p + 1, N_SLOTS - 1) + 1):
                    ch = wpool.tile([128, CW], f32, tag="ch")
                    if i == 2 * p + 1:
                        # strip starts at the pair's second half
                        nc.scalar.activation(
                            ch[:, :1024], ps[:, 1024:2048],
                            mybir.ActivationFunctionType.Tanh,
                            bias=si_sb[:, i:i + 1],
                        )
                        nc.vector.tensor_mul(ch[:, :128], ch[:, :128],
                                             mask_sb[:, :])
                        nc.sync.dma_start(outs[i][:, 0:1024], ch[:, :1024])
                    else:
                        nc.scalar.activation(
                            ch[:, :], ps[:, :],
                            mybir.ActivationFunctionType.Tanh,
                            bias=si_sb[:, i:i + 1],
                        )
                        if i == 2 * p:
                            # strip starts at the pair's first half
                            nc.vector.tensor_mul(ch[:, :128], ch[:, :128],
                                                 mask_sb[:, :])
                        off = CW * p - 1024 * i
                        nc.sync.dma_start(outs[i][:, off:off + CW], ch[:, :])
    nc.compile()
    _NC_CACHE["nc"] = nc
    return nc


_LAST_RESULTS = {}


def _make_in_maps(si32, sj32, raw):
    from ml_dtypes import bfloat16

    sj_pad = np.zeros(N + 128 * (N_CORES - 1), F32)
    sj_pad[:N] = sj32
    si_groups = si32.reshape(N // 128, 128)  # [64, 128]
    mask = np.triu(np.ones((128, 128), F32), k=1)

    in_maps = []
    for k in range(N_CORES):
        simask = np.empty((128, N_SLOTS + 128), F32)
        simask[:, :N_SLOTS] = si_groups[
            [N_SLOTS * i + k for i in range(N_SLOTS)]].T  # [128, 8]
        simask[:, N_SLOTS:] = mask
        sj_core = sj_pad[128 * k:128 * k + N]
        hi = sj_core.astype(bfloat16)
        lo = (sj_core - hi.astype(F32)).astype(bfloat16)
        if raw:
            sjhl = np.ones((2, N + 128), bfloat16)
            sjhl[0, :N] = hi
            sjhl[1, :N] = lo
        else:
            sjhl = np.ascontiguousarray(np.stack([hi, lo]))
        in_maps.append({"sjhl": sjhl, "simask": simask})
    return in_maps


def _use_raw():
    import os
    return os.environ.get("KERNEL_IMPL", "raw") == "raw"


def kernel(**inputs):
    _install_ntff_hook_shim()
    from concourse import bass_utils

    si, sj = _host_scores(inputs, np.float64)
    si32 = np.ascontiguousarray(si, dtype=F32)
    sj32 = np.ascontiguousarray(sj, dtype=F32)
    raw = _use_raw()
    in_maps = _make_in_maps(si32, sj32, raw)

    nc = _build_nc_raw() if raw else _build_nc()
    res = bass_utils.run_bass_kernel_spmd(
        nc, in_maps, core_ids=list(range(N_CORES)))
    _LAST_RESULTS["res"] = res

    full = np.zeros((N, N), F32)
    for k in range(N_CORES):
        for i in range(N_SLOTS):
            g = N_SLOTS * i + k
            r0 = 128 * g
            wv = N - r0
            full[r0:r0 + 128, r0:N] = res.results[k][f"out{i}"][:, :wv]
    return full


# revision 17
# speedup vs baseline: 1.1382x; 1.1382x over previous
"""Trainium2 kernel for nn_DependencyParsingNetwork.

Pipeline:
  * Host (numpy, float64): embedding lookup + 2-layer BiLSTM (inherently
    sequential, tiny) reduced to two length-N score vectors
    si = x @ w[:2H] + b, sj = x @ w[2H:], so that
    out[r, c] = tanh(si[r] + sj[c]) for c > r, else 0.
  * Device (raw Bass, 8 NeuronCores): the memory-bound N x N upper-triangular
    tanh outer-sum. Row-groups of 128 are dealt round-robin to cores
    (group g -> core g % 8, slot g // 8). Each slot computes only the strip
    starting at its own diagonal, so every core writes ~19 MB instead of 32 MB
    and the zero lower-triangle is never written on device.
    sj is split hi/lo into two bf16 rows so a single-pass K=2 matmul against
    a ones stationary reconstructs sj broadcast across all 128 partitions
    into PSUM (error ~2^-18; a fp32 matmul would need 2 PE passes). Each
    2048-col PSUM pair is consumed by every slot whose strip covers it: one
    ScalarE activation tanh(psum + si_bias) -> SBUF per slot, the 128x128
    diagonal tile masked on VectorE, then a 1 MiB DMA out. Hand-rolled
    semaphores (no TileContext) keep the preamble/tail small; an 8-deep SBUF
    chunk ring with per-slot DMA-completion sems keeps the output stream at
    HBM line rate (~420 GB/s observed per core).
    A single SPMD program works for all cores because each core receives
    sj shifted by 128*k, which makes strip offsets uniform (1024*i).
  * Host: scatter the 64 strips into the zero-initialized full output.
"""

import sys
import types

import numpy as np

N = 8192
H = 50
N_CORES = 8
N_SLOTS = 8  # row-groups of 128 per core
F32 = np.float32


# ---------------------------------------------------------------------------
# Host math: embedding + BiLSTM -> si, sj
# ---------------------------------------------------------------------------

def _lstm_pair(x, wf, hf, bf, wb, hb, bb, dtype):
    """Fwd+bwd LSTM over x [n, d] -> concat [n, 2H]. The two directions are
    batched into one python loop (fwd at step t, bwd at step n-1-t)."""
    n = x.shape[0]
    g_in = np.stack([
        x @ wf.T.astype(dtype) + bf.astype(dtype),
        x @ wb.T.astype(dtype) + bb.astype(dtype),
    ])  # [2, n, 4H]
    whhT = np.stack([hf.T.astype(dtype), hb.T.astype(dtype)])  # [2, H, 4H]
    h = np.zeros((2, H), dtype)
    c = np.zeros((2, H), dtype)
    out_f = np.empty((n, H), dtype)
    out_b = np.empty((n, H), dtype)
    for t in range(n):
        tb = n - 1 - t
        g = np.einsum("dk,dkg->dg", h, whhT)
        g[0] += g_in[0, t]
        g[1] += g_in[1, tb]
        i = 1.0 / (1.0 + np.exp(-g[:, :H]))
        f = 1.0 / (1.0 + np.exp(-g[:, H:2 * H]))
        gg = np.tanh(g[:, 2 * H:3 * H])
        o = 1.0 / (1.0 + np.exp(-g[:, 3 * H:]))
        c = f * c + i * gg
        h = o * np.tanh(c)
        out_f[t] = h[0]
        out_b[tb] = h[1]
    return np.concatenate([out_f, out_b], axis=1)


def _host_scores(inp, dtype=np.float64):
    emb = inp["emb"].astype(dtype)
    tok = np.asarray(inp["token_vector"]).reshape(-1).astype(np.int64)
    x = np.concatenate([emb[tok], inp["pos_vector"].astype(dtype)], axis=1)
    x = _lstm_pair(x, inp["w_ih_l0_fwd"], inp["w_hh_l0_fwd"], inp["b_l0_fwd"],
                   inp["w_ih_l0_bwd"], inp["w_hh_l0_bwd"], inp["b_l0_bwd"], dtype)
    x = _lstm_pair(x, inp["w_ih_l1_fwd"], inp["w_hh_l1_fwd"], inp["b_l1_fwd"],
                   inp["w_ih_l1_bwd"], inp["w_hh_l1_bwd"], inp["b_l1_bwd"], dtype)
    mlp_w = inp["mlp_w"].astype(dtype)
    si = x @ mlp_w[0, :2 * H] + inp["mlp_b"].astype(dtype)[0]
    sj = x @ mlp_w[0, 2 * H:]
    return si, sj


# ---------------------------------------------------------------------------
# Device kernel
# ---------------------------------------------------------------------------

def _install_ntff_hook_shim():
    """The agent image's antenv stub lacks axon_hooks; run_bass_kernel_spmd
    imports it when tracing is requested. Provide it if missing."""
    try:
        import antenv.axon_hooks  # noqa: F401
        return
    except ImportError:
        pass
    try:
        from trn_agent_boot.trn_boot import _ntff_profile_via_ctypes
        hook = _ntff_profile_via_ctypes("/opt/axon/libaxon_pjrt.so")
    except Exception:
        hook = None
    mod = types.ModuleType("antenv.axon_hooks")
    mod.get_axon_ntff_profile_hook = lambda: hook
    mod.set_axon_ntff_profile_hook = lambda h: None
    sys.modules["antenv.axon_hooks"] = mod


_NC_CACHE = {}


def _strip_width(i):
    return N - 1024 * i


def _chunks():
    """Global chunk order: (pair p, slot i, half?). Pair p covers local sj
    columns [2048p, 2048p+2048). Slot i uses pair p iff i <= 2p+1; the chunk
    is the pair's second half only when i == 2p+1, and the strip's first 128
    columns need the triangular mask when i in (2p, 2p+1)."""
    out = []
    for p in range(N // 2048):
        for i in range(min(2 * p + 1, N_SLOTS - 1) + 1):
            out.append((p, i, i == 2 * p + 1))
    return out


def _build_nc_raw():
    """Raw bacc implementation (no TileContext): hand-rolled semaphores,
    minimal preamble/tail."""
    if "nc_raw" in _NC_CACHE:
        return _NC_CACHE["nc_raw"]
    import concourse.bacc as bacc
    import concourse.mybir as mybir

    f32 = mybir.dt.float32
    bf16 = mybir.dt.bfloat16
    CW = 2048
    nc = bacc.Bacc("TRN2", target_bir_lowering=False, debug=False,
                   num_devices=N_CORES)
    # row 0: sj_hi ++ ones(128); row 1: sj_lo ++ ones(128)
    sj_d = nc.dram_tensor("sjhl", [2, N + 128], bf16, kind="ExternalInput")
    sim_d = nc.dram_tensor("simask", [128, N_SLOTS + 128], f32,
                           kind="ExternalInput")
    outs = [
        nc.dram_tensor(f"out{i}", [128, _strip_width(i)], f32,
                       kind="ExternalOutput")
        for i in range(N_SLOTS)
    ]

    chunks = _chunks()
    n_chunks = len(chunks)
    # masked chunk ordinal (prefix count) for each chunk index
    masked = [(i == 2 * p or half) for (p, i, half) in chunks]
    mord = [sum(masked[:n + 1]) for n in range(n_chunks)]
    cum_chunks = []  # chunks through pair p
    c = 0
    for p in range(N // CW):
        c += min(2 * p + 1, N_SLOTS - 1) + 1
        cum_chunks.append(c)

    NCH = 8  # ch ring depth

    with (
        nc.sbuf_tensor("sjhl_sb", [2, N + 128], bf16) as sj_sb,
        nc.sbuf_tensor("simask_sb", [128, N_SLOTS + 128], f32) as sim_sb,
        nc.sbuf_tensor("ch_sb", [128, NCH * CW], f32) as ch_sb,
        nc.psum_tensor("ps0", [128, CW], f32) as ps0,
        nc.psum_tensor("ps1", [128, CW], f32) as ps1,
    ):
        pss = [ps0, ps1]
        s_in_sj = nc.alloc_semaphore("s_in_sj")
        s_in_sim = nc.alloc_semaphore("s_in_sim")
        s_mm = nc.alloc_semaphore("s_mm")
        s_act = nc.alloc_semaphore("s_act")
        s_dve = nc.alloc_semaphore("s_dve")
        # per-ch-ring-slot DMA completion sems: slot j's DMAs are serialized
        # by the ACT->DMA->ACT chain, so each sem unambiguously counts its
        # own slot's completed DMAs (a shared counter would interleave
        # per-SDMA-engine increments of concurrent DMAs).
        s_slot = [nc.alloc_semaphore(f"s_slot{j}") for j in range(8)]
        my_sems = [s_in_sj, s_in_sim, s_mm, s_act, s_dve] + s_slot

        # Sems are zero at program start: the NEFF-level teardown emitted by
        # the compiler resets the whole semaphore space at the end of every
        # NEFF, so (like Tile programs) we rely on that convention instead of
        # spending a clear + all-engine barrier before the first DMA.
        del my_sems

        # Work records: one per (ACT, DMA) unit. Chunk 0 is split 512+1536 so
        # the first output DMA launches after a single matmul. Fields:
        # p: psum pair, i: slot, src: cols within pss[p%2], ch: (ring slot,
        # lo, hi) within that slot's ch region, dst: cols within out_i,
        # mm: s_mm threshold, msk: apply triangular mask (dst starts at the
        # strip diagonal in every masked record).
        recs = []
        for n, (p, i, half) in enumerate(chunks):
            sl = n % NCH
            if n == 0:
                recs.append((p, i, (0, 512), (sl, 0, 512), (0, 512), 1, True, n))
                recs.append((p, i, (512, CW), (sl, 512, CW), (512, CW), 4, False, n))
            elif half:
                recs.append((p, i, (1024, CW), (sl, 0, 1024), (0, 1024),
                             4 * (p + 1), True, n))
            else:
                off = CW * p - 1024 * i
                recs.append((p, i, (0, CW), (sl, 0, CW), (off, off + CW),
                             4 * (p + 1), i == 2 * p, n))

        # s_slot[s] DMA-increments contributed by chunks before chunk n
        dmas_of_chunk = [sum(1 for r in recs if r[7] == n) for n in range(n_chunks)]
        # ACT of chunk n (n>=NCH) must see all DMAs of chunks {n%NCH, .., n-NCH}
        act_slot_need = [0] * n_chunks
        for n in range(NCH, n_chunks):
            act_slot_need[n] = 16 * sum(dmas_of_chunk[m] for m in range(n % NCH, n - NCH + 1, NCH))
        mask_ord = [sum(1 for r2 in recs[:r + 1] if r2[6]) for r in range(len(recs))]
        recs_through_pair = [sum(1 for r in recs if r[0] <= p) for p in range(N // CW)]
        slot_total = [16 * sum(1 for r in recs if r[3][0] == j) for j in range(NCH)]

        with nc.Block() as block:

            @block.sync
            def _(sync):
                sync.dma_start(sj_sb.ap(), sj_d[:]).then_inc(s_in_sj, 16)
                sync.dma_start(sim_sb.ap(), sim_d[:]).then_inc(s_in_sim, 16)
                for r, (p, i, srcc, (sl, clo, chi), (dlo, dhi), mm, msk, n) in enumerate(recs):
                    if msk:
                        sync.wait_ge(s_dve, mask_ord[r])
                    else:
                        sync.wait_ge(s_act, r + 1)
                    sync.dma_start(
                        outs[i][:, dlo:dhi],
                        ch_sb[:, sl * CW + clo:sl * CW + chi],
                    ).then_inc(s_slot[sl], 16)
                for j in range(NCH):
                    sync.wait_ge(s_slot[j], slot_total[j])

            @block.tensor
            def _(tensor):
                tensor.wait_ge(s_in_sj, 16)
                for p in range(N // CW):
                    if p >= 2:
                        tensor.wait_ge(s_act, recs_through_pair[p - 2])
                    for q in range(CW // 512):
                        lo = CW * p + 512 * q
                        tensor.matmul(
                            pss[p % 2][:, 512 * q:512 * (q + 1)],
                            sj_sb[0:2, N:N + 128],
                            sj_sb[0:2, lo:lo + 512],
                        ).then_inc(s_mm, 1)

            @block.scalar
            def _(scalar):
                scalar.wait_ge(s_in_sim, 16)
                prev_chunk = -1
                for r, (p, i, (slo, shi), (sl, clo, chi), dst, mm, msk, n) in enumerate(recs):
                    scalar.wait_ge(s_mm, mm)
                    if n != prev_chunk and act_slot_need[n]:
                        scalar.wait_ge(s_slot[sl], act_slot_need[n])
                    prev_chunk = n
                    scalar.activation(
                        ch_sb[:, sl * CW + clo:sl * CW + chi],
                        pss[p % 2][:, slo:shi],
                        mybir.ActivationFunctionType.Tanh,
                        bias=sim_sb[:, i:i + 1],
                    ).then_inc(s_act, 1)

            @block.vector
            def _(vector):
                for r, (p, i, srcc, (sl, clo, chi), dst, mm, msk, n) in enumerate(recs):
                    if not msk:
                        continue
                    vector.wait_ge(s_act, r + 1)
                    vector.tensor_mul(
                        ch_sb[:, sl * CW:sl * CW + 128],
                        ch_sb[:, sl * CW:sl * CW + 128],
                        sim_sb[:, N_SLOTS:],
                    ).then_inc(s_dve, 1)

    nc.compile()
    _NC_CACHE["nc_raw"] = nc
    return nc


def _build_nc():
    if "nc" in _NC_CACHE:
        return _NC_CACHE["nc"]
    import concourse.bacc as bacc
    import concourse.mybir as mybir
    import concourse.tile as tile

    f32 = mybir.dt.float32
    bf16 = mybir.dt.bfloat16
    nc = bacc.Bacc("TRN2", target_bir_lowering=False, debug=False,
                   num_devices=N_CORES)
    # sj split into bf16 hi+lo rows so a single-pass K=2 bf16 matmul against
    # a ones stationary reconstructs sj (error ~2^-18) broadcast across all
    # 128 partitions: psum[p, c] = 1*hi[c] + 1*lo[c].
    sj_d = nc.dram_tensor("sjhl", [2, N], bf16, kind="ExternalInput")
    # si ([:, :8]) and the 128x128 triangular mask ([:, 8:136]) share one DMA
    sim_d = nc.dram_tensor("simask", [128, N_SLOTS + 128], f32,
                           kind="ExternalInput")
    outs = [
        nc.dram_tensor(f"out{i}", [128, _strip_width(i)], f32,
                       kind="ExternalOutput")
        for i in range(N_SLOTS)
    ]

    CW = 2048  # PSUM chunk width: 4 banks; one broadcast serves many slots

    with tile.TileContext(nc) as tc:
        with (
            tc.tile_pool(name="const", bufs=1) as cpool,
            tc.tile_pool(name="work", bufs=8) as wpool,
            tc.tile_pool(name="psum", bufs=2, space="PSUM") as ppool,
        ):
            sj_sb = cpool.tile([2, N], bf16)
            nc.sync.dma_start(sj_sb[:], sj_d[:])
            sim_sb = cpool.tile([128, N_SLOTS + 128], f32)
            nc.sync.dma_start(sim_sb[:], sim_d[:])
            si_sb = sim_sb[:, :N_SLOTS]
            mask_sb = sim_sb[:, N_SLOTS:]
            ones_sb = cpool.tile([2, 128], bf16)
            nc.gpsimd.memset(ones_sb[:], 1.0)

            # Pair p covers local sj columns [2048p, 2048p+2048): the 1024-col
            # ranges r=2p (first half) and r=2p+1 (second half). Slot i's strip
            # starts at local col 1024i, so slot i uses pair p iff i <= 2p+1.
            for p in range(N // CW):
                ps = ppool.tile([128, CW], f32, tag="ps")
                for q in range(CW // 512):
                    lo = CW * p + 512 * q
                    nc.tensor.matmul(
                        ps[:, 512 * q:512 * (q + 1)],
                        ones_sb[:, :],
                        sj_sb[0:2, lo:lo + 512],
                    )
                for i in range(min(2 * p + 1, N_SLOTS - 1) + 1):
                    ch = wpool.tile([128, CW], f32, tag="ch")
                    if i == 2 * p + 1:
                        # strip starts at the pair's second half
                        nc.scalar.activation(
                            ch[:, :1024], ps[:, 1024:2048],
                            mybir.ActivationFunctionType.Tanh,
                            bias=si_sb[:, i:i + 1],
                        )
                        nc.vector.tensor_mul(ch[:, :128], ch[:, :128],
                                             mask_sb[:, :])
                        nc.sync.dma_start(outs[i][:, 0:1024], ch[:, :1024])
                    else:
                        nc.scalar.activation(
                            ch[:, :], ps[:, :],
                            mybir.ActivationFunctionType.Tanh,
                            bias=si_sb[:, i:i + 1],
                        )
                        if i == 2 * p:
                            # strip starts at the pair's first half
                            nc.vector.tensor_mul(ch[:, :128], ch[:, :128],
                                                 mask_sb[:, :])
                        off = CW * p - 1024 * i
                        nc.sync.dma_start(outs[i][:, off:off + CW], ch[:, :])
    nc.compile()
    _NC_CACHE["nc"] = nc
    return nc


_LAST_RESULTS = {}


def _make_in_maps(si32, sj32, raw):
    from ml_dtypes import bfloat16

    sj_pad = np.zeros(N + 128 * (N_CORES - 1), F32)
    sj_pad[:N] = sj32
    si_groups = si32.reshape(N // 128, 128)  # [64, 128]
    mask = np.triu(np.ones((128, 128), F32), k=1)

    in_maps = []
    for k in range(N_CORES):
        simask = np.empty((128, N_SLOTS + 128), F32)
        simask[:, :N_SLOTS] = si_groups[
            [N_SLOTS * i + k for i in range(N_SLOTS)]].T  # [128, 8]
        simask[:, N_SLOTS:] = mask
        sj_core = sj_pad[128 * k:128 * k + N]
        hi = sj_core.astype(bfloat16)
        lo = (sj_core - hi.astype(F32)).astype(bfloat16)
        if raw:
            sjhl = np.ones((2, N + 128), bfloat16)
            sjhl[0, :N] = hi
            sjhl[1, :N] = lo
        else:
            sjhl = np.ascontiguousarray(np.stack([hi, lo]))
        in_maps.append({"sjhl": sjhl, "simask": simask})
    return in_maps


def _use_raw():
    import os
    return os.environ.get("KERNEL_IMPL", "raw") == "raw"


def kernel(**inputs):
    _install_ntff_hook_shim()
    from concourse import bass_utils

    si, sj = _host_scores(inputs, np.float64)
    si32 = np.ascontiguousarray(si, dtype=F32)
    sj32 = np.ascontiguousarray(sj, dtype=F32)
    raw = _use_raw()
    in_maps = _make_in_maps(si32, sj32, raw)

    nc = _build_nc_raw() if raw else _build_nc()
    try:
        res = bass_utils.run_bass_kernel_spmd(
            nc, in_maps, core_ids=list(range(N_CORES)))
    except Exception:
        # transient PJRT/runtime hiccups recover on retry
        res = bass_utils.run_bass_kernel_spmd(
            nc, in_maps, core_ids=list(range(N_CORES)))
    _LAST_RESULTS["res"] = res

    full = np.zeros((N, N), F32)
    for k in range(N_CORES):
        for i in range(N_SLOTS):
            g = N_SLOTS * i + k
            r0 = 128 * g
            wv = N - r0
            full[r0:r0 + 128, r0:N] = res.results[k][f"out{i}"][:, :wv]
    return full
